# revision 24
# baseline (speedup 1.0000x reference)
"""TransformerXL attention (AttentionXL) Bass kernel for Trainium2, 8 NeuronCores.

Sharding: pure data-parallel over batch (BS=8 -> 1 batch element per core).
All weights replicated per core; no collectives.

v2 design (multiplicative softmax split, unnormalized attention):
  exp(s*(C+S)) = exp(s*C) * exp(s*S):
    expC  = exp(s*C) on ScalarE straight from the C psum
    expS  = exp(s*P) on ScalarE (this replaces the fp32->bf16 cast of the
            position scores that the DMA rel-shift roundtrip needed anyway)
    A~    = expC (*) expS_sheared   fused with Z-rowsum on DVE
            (tensor_tensor_reduce, bf16 fast path)
  A~ stays unnormalized through transpose (PE) + AV; 1/Z is folded into the
  AVT psum->sbuf copy as a per-head broadcast multiply.
  Causal mask: fill expS with 0 on masked cols (gpsimd affine_select).

  C and P score matmuls are head-pair row-packed (tile_position (0,0)/(64,0),
  contraction 64 each -> concurrent).  P scores m-trimmed to m >= 384-128*ib.
  Projections K/R stream per head-pair chunk interleaved with attention so
  ScalarE/DVE softmax work overlaps PE projection work.
"""

import os
import sys

for _p in (
    "/root/.axon_site",
    "/root/.axon_site/_ro/trn_rl_repo",
    "/root/.axon_site/_ro/pypackages",
    "/opt/trn_rl_repo",
):
    if os.path.isdir(_p) and _p not in sys.path:
        sys.path.append(_p)

import numpy as np
import ml_dtypes

import concourse.bass as bass
import concourse.mybir as mybir
import concourse.tile as tile
from concourse.bass_utils import run_bass_kernel_spmd
from concourse.masks import make_identity

BF16 = mybir.dt.bfloat16
FP32 = mybir.dt.float32
AF = mybir.ActivationFunctionType
ALU = mybir.AluOpType
nbf16 = ml_dtypes.bfloat16

CUR, FULL, BS, DIM, H, D = 512, 1024, 8, 1024, 16, 64
PREV = FULL - CUR
SCALE = 1.0 / D**0.5
P = 128
NIB = CUR // P    # 4 query blocks
NJC = FULL // P   # 8 key chunks
NCH = DIM // P    # 8 dim chunks
NHP = H // 2      # 8 head pairs

_BUILT = None


def _split_multiwait(nc):
    """encode at most ONE sync wait per TPB instruction (single wait slot):
    prepend same-engine NoOps carrying extra waits."""
    n_split = 0
    for fn in nc.m.functions:
        for blk in fn.blocks:
            insts = list(blk.instructions)
            out = []
            for ins in insts:
                si = ins.sync_info
                if si is not None and si.on_wait and len(si.on_wait) > 1:
                    waits = list(si.on_wait)
                    for w in waits[:-1]:
                        nop = mybir.InstNoOp(
                            name=f"{ins.name}-ws{n_split}",
                            engine=ins.engine,
                            sync_info=mybir.SyncInfo(on_wait=[w], on_update=[]),
                            text_hint="waitsplit",
                        )
                        out.append(nop)
                        n_split += 1
                    ins.sync_info = mybir.SyncInfo(
                        on_wait=[waits[-1]],
                        on_update=list(si.on_update or []),
                    )
                out.append(ins)
            blk.instructions = out
    return n_split


def _build(split_waits=True):
    nc = bass.Bass()

    # activations transposed: [X^T | Xc^T | Pos^T] cols
    acts = nc.declare_dram_parameter("acts", [DIM, FULL + CUR + FULL], BF16, isOutput=False)
    # weights: [W_q | W_pos | W_k | W_v] cols
    wmats = nc.declare_dram_parameter("wmats", [DIM, 4 * DIM], BF16, isOutput=False)
    wproj = nc.declare_dram_parameter("wproj", [DIM, DIM], BF16, isOutput=False)
    # biases: [p, 4*NCH] = qu | dqv(=v-u) | k | pos chunks
    biases = nc.declare_dram_parameter("biases", [P, 4 * NCH], FP32, isOutput=False)
    bout = nc.declare_dram_parameter("bout", [DIM], FP32, isOutput=False)
    out = nc.declare_dram_parameter("out", [CUR, DIM], FP32, isOutput=True)

    with tile.TileContext(nc) as tc:
        from contextlib import ExitStack

        with ExitStack() as ctx:
            persist = ctx.enter_context(tc.tile_pool(name="persist", bufs=1))

            QuT = persist.tile([P, NCH, CUR], BF16, tag="QuT")
            QvT = persist.tile([P, NCH, CUR], BF16, tag="QvT")
            V = persist.tile([P, NJC, DIM], BF16, tag="V")
            AVT = persist.tile([P, NCH, CUR], BF16, tag="AVT")
            xcT = persist.tile([P, NCH, CUR], BF16, tag="xcT")
            xT = persist.tile([P, NCH, FULL], BF16, tag="xT")
            pT = persist.tile([P, NCH, FULL], BF16, tag="pT")
            bias_t = persist.tile([P, 4, NCH], FP32, tag="bias_t")
            boutb = persist.tile([P, DIM], FP32, tag="boutb")
            ident = persist.tile([P, P], BF16, tag="ident")

            make_identity(nc, ident)
            zero_reg = nc.gpsimd.to_reg(0.0)
            nc.sync.dma_start(bias_t, biases.rearrange("p (b c) -> p b c", b=4))
            # bout broadcast: replicate via DMA into all 128 partition rows
            nc.sync.dma_start(
                boutb,
                bout[None, :].broadcast_to((P, DIM)),
            )

            # ---- input DMAs, finest-grain-first so compute starts early ----
            nc.sync.dma_start(xcT, acts[:, FULL:FULL + CUR].rearrange("(c p) f -> p c f", p=P))

            # weight streaming pools
            wq_pool = ctx.enter_context(tc.tile_pool(name="wq", bufs=2))
            wkr_pool = ctx.enter_context(tc.tile_pool(name="wkr", bufs=2))
            wv_pool = ctx.enter_context(tc.tile_pool(name="wv", bufs=2))
            ktr_pool = ctx.enter_context(tc.tile_pool(name="ktr", bufs=2))
            qv_pool = None  # QvT persistent

            # psum pools
            scores = ctx.enter_context(tc.tile_pool(name="scores", bufs=2, space="PSUM"))
            krp = ctx.enter_context(tc.tile_pool(name="krp", bufs=2, space="PSUM"))
            tps = ctx.enter_context(tc.tile_pool(name="tps", bufs=1, space="PSUM"))
            avp = ctx.enter_context(tc.tile_pool(name="avp", bufs=1, space="PSUM"))

            # attention work pools
            work = ctx.enter_context(tc.tile_pool(name="work", bufs=1))
            dram = ctx.enter_context(tc.tile_pool(name="dram", bufs=4, space="DRAM"))

            # ---------------- Q projection ----------------
            for oc in range(NCH):
                wq_t = wq_pool.tile([P, NCH, P], BF16, tag="wq", name="wq_t")
                nc.sync.dma_start(
                    wq_t,
                    wmats[:, oc * P:(oc + 1) * P].rearrange("(c p) f -> p c f", p=P),
                )
                qps = krp.tile([P, 512], FP32, tag="kr", name="qps")
                for kc in range(NCH):
                    nc.tensor.matmul(
                        qps, wq_t[:, kc, :], xcT[:, kc, :],
                        start=(kc == 0), stop=(kc == NCH - 1),
                    )
                nc.scalar.activation(
                    QuT[:, oc, :], qps, AF.Identity,
                    bias=bias_t[:, 0, oc:oc + 1],
                )
                nc.gpsimd.tensor_scalar_add(
                    QvT[:, oc, :], QuT[:, oc, :], bias_t[:, 1, oc:oc + 1]
                )

            nc.sync.dma_start(xT, acts[:, 0:FULL].rearrange("(c p) f -> p c f", p=P))
            nc.sync.dma_start(
                pT, acts[:, FULL + CUR:].rearrange("(c p) f -> p c f", p=P))

            # ---- software-pipelined head-pair loop ----
            # iteration i: scores(i) [C/P mm + exp + shear-dma + mask + STT],
            #              K/R proj(i+1), V chunk (i<2), transposes(i-1), AV(i-2)
            KR = {}
            ST = {}
            AT = {}

            def kr_quarter(hp, q):
                # q: 0=K jh0, 1=K jh1, 2=R jh0, 3=R jh1
                if q == 0:
                    KT_t = ktr_pool.tile([P, FULL], BF16, tag="kt", name="KT_t")
                    RT_t = ktr_pool.tile([P, FULL], BF16, tag="rt", name="RT_t")
                    wk_t = wkr_pool.tile([P, NCH, P], BF16, tag="wk", name="wk_t")
                    wp_t = wkr_pool.tile([P, NCH, P], BF16, tag="wp", name="wp_t")
                    nc.sync.dma_start(
                        wk_t,
                        wmats[:, 2 * DIM + hp * P:2 * DIM + (hp + 1) * P].rearrange(
                            "(c p) f -> p c f", p=P),
                    )
                    nc.sync.dma_start(
                        wp_t,
                        wmats[:, DIM + hp * P:DIM + (hp + 1) * P].rearrange(
                            "(c p) f -> p c f", p=P),
                    )
                    KR[hp] = (KT_t, RT_t)
                    kr_quarter.w = (wk_t, wp_t)
                KT_t, RT_t = KR[hp]
                wk_t, wp_t = kr_quarter.w
                jh = q % 2
                sl = slice(jh * 512, (jh + 1) * 512)
                if q < 2:
                    kps = krp.tile([P, 512], FP32, tag="kr", name="kps")
                    for kc in range(NCH):
                        nc.tensor.matmul(
                            kps, wk_t[:, kc, :], xT[:, kc, sl],
                            start=(kc == 0), stop=(kc == NCH - 1),
                        )
                    nc.scalar.activation(KT_t[:, sl], kps, AF.Identity,
                                         bias=bias_t[:, 2, hp:hp + 1])
                else:
                    rps = krp.tile([P, 512], FP32, tag="kr", name="rps")
                    for kc in range(NCH):
                        nc.tensor.matmul(
                            rps, wp_t[:, kc, :], pT[:, kc, sl],
                            start=(kc == 0), stop=(kc == NCH - 1),
                        )
                    nc.vector.tensor_scalar_add(RT_t[:, sl], rps,
                                                bias_t[:, 3, hp:hp + 1])

            def scores_init(hp):
                pd = [dram.tile([CUR, FULL], BF16, tag=f"pd{hh}", name=f"pd{hh}")
                      for hh in range(2)]
                zc = work.tile([P, 8], FP32, tag="zc", bufs=2, name="zc")
                ST[hp] = (KR.pop(hp), pd, zc, {})

            def scores_ib(hp, ib):
                (KT_t, RT_t), pd, zc, a_sbs = ST[hp]
                isl = slice(ib * P, (ib + 1) * P)
                jmax = 640 + ib * P
                mlo = PREV - P - ib * P
                cps_, pps_, acs, ss = [], [], [], []
                for hh in range(2):
                    cps_.append(scores.tile([P, FULL], FP32, tag="sc", name="cp"))
                for jh in range(2):
                    j0, j1 = jh * 512, min((jh + 1) * 512, jmax)
                    for hh in range(2):
                        rs = slice(hh * D, (hh + 1) * D)
                        nc.tensor.matmul(
                            cps_[hh][:, j0:j1],
                            QuT[rs, hp, isl], KT_t[rs, j0:j1],
                            start=True, stop=True,
                            tile_position=(hh * D, 0),
                        )
                for hh in range(2):
                    a_c = work.tile([P, FULL], BF16, tag="ac", bufs=4, name="a_c")
                    nc.scalar.activation(
                        a_c[:, 0:jmax], cps_[hh][:, 0:jmax], AF.Exp, scale=SCALE,
                    )
                    acs.append(a_c)
                for hh in range(2):
                    pps_.append(scores.tile([P, FULL], FP32, tag="sc", name="pp"))
                for mh in range(2):
                    m0, m1 = max(mh * 512, mlo), (mh + 1) * 512
                    if m1 <= m0:
                        continue
                    for hh in range(2):
                        rs = slice(hh * D, (hh + 1) * D)
                        nc.tensor.matmul(
                            pps_[hh][:, m0:m1],
                            QvT[rs, hp, isl], RT_t[rs, m0:m1],
                            start=True, stop=True,
                            tile_position=(hh * D, 0),
                        )
                for hh in range(2):
                    p_sb = work.tile([P, FULL], BF16, tag="psb", bufs=3, name="p_sb")
                    nc.scalar.activation(
                        p_sb[:, mlo:], pps_[hh][:, mlo:], AF.Exp, scale=SCALE,
                    )
                    nc.sync.dma_start(
                        bass.AP(tensor=pd[hh].tensor,
                                offset=pd[hh].offset + ib * P * FULL + mlo,
                                ap=[[FULL, P], [1, FULL - mlo]]),
                        p_sb[:, mlo:],
                    )
                    s_sb = work.tile([P, FULL], BF16, tag="ssb", bufs=4, name="s_sb")
                    nc.sync.dma_start(
                        s_sb[:, 0:jmax],
                        bass.AP(tensor=pd[hh].tensor,
                                offset=pd[hh].offset + ib * P * (FULL - 1) + (PREV - 1),
                                ap=[[FULL - 1, P], [1, jmax]]),
                    )
                    nc.gpsimd.affine_select(
                        out=s_sb[:, 512:jmax], in_=s_sb[:, 512:jmax],
                        compare_op=ALU.is_ge, fill=zero_reg,
                        base=ib * P, channel_multiplier=1,
                        pattern=[[-1, jmax - 512]],
                    )
                    ss.append(s_sb)
                for hh in range(2):
                    a_sb = work.tile([P, 640 + 128 * ib], BF16,
                                     tag=f"asb{ib}", bufs=4, name="a_sb")
                    nc.vector.scalar_tensor_tensor(
                        out=a_sb, in0=acs[hh][:, 0:jmax],
                        scalar=1.0, in1=ss[hh][:, 0:jmax],
                        op0=ALU.mult, op1=ALU.mult,
                        accum_out=zc[:, hh * 4 + ib:hh * 4 + ib + 1],
                    )
                    a_sbs[(hh, ib)] = a_sb

            def transpose_hh(hp, hh):
                _, _, zc, a_sbs = ST[hp]
                if hh == 0:
                    a_t = [work.tile([P, NJC, CUR], BF16, tag=f"at{h2}", bufs=2,
                                     name=f"at{h2}") for h2 in range(2)]
                    rz8 = work.tile([P, 8], FP32, tag="rz8", bufs=2, name="rz8")
                    nc.vector.reciprocal(rz8, zc)
                    AT[hp] = a_t
                    transpose_hh.rz8 = rz8
                a_t = AT[hp]
                rz8 = transpose_hh.rz8
                for ib in range(NIB):
                    isl = slice(ib * P, (ib + 1) * P)
                    njc_v = min(ib + 5, NJC)
                    a_sb = a_sbs[(hh, ib)]
                    nc.vector.tensor_scalar_mul(
                        a_sb, a_sb, rz8[:, hh * 4 + ib:hh * 4 + ib + 1]
                    )
                    for tg in range(2):
                        jcs = [j for j in range(tg * 4, min((tg + 1) * 4, njc_v))]
                        if not jcs:
                            continue
                        tp = tps.tile([P, 4, P], BF16, tag="tp", name="tp")
                        for k, jc in enumerate(jcs):
                            nc.tensor.transpose(
                                tp[:, k], a_sb[:, jc * P:(jc + 1) * P], ident
                            )
                        nc.vector.tensor_copy(
                            a_t[hh][:, jcs[0]:jcs[0] + len(jcs), isl],
                            tp[:, :len(jcs)],
                        )
                if hh == 1:
                    ST.pop(hp)

            def av_block(hp):
                at_pair = AT.pop(hp)
                av = avp.tile([P, CUR], FP32, tag="av", name="av")
                for jc in range(NJC):
                    ilo = max(0, (jc - 4)) * P
                    for hh in range(2):
                        h = 2 * hp + hh
                        nc.tensor.matmul(
                            av[hh * D:(hh + 1) * D, ilo:],
                            V[:, jc, h * D:(h + 1) * D],
                            at_pair[hh][:, jc, ilo:],
                            start=(jc == 0), stop=(jc == NJC - 1),
                            tile_position=(0, hh * D),
                        )
                nc.vector.tensor_copy(AVT[:, hp, :], av)

            def v_piece(mh, jc):
                sl = slice(mh * 512, (mh + 1) * 512)
                if jc == 0:
                    wv_t = wv_pool.tile([P, NCH, 512], BF16, tag="wv", name="wv_t")
                    nc.sync.dma_start(
                        wv_t,
                        wmats[:, 3 * DIM + mh * 512:3 * DIM + (mh + 1) * 512].rearrange(
                            "(c p) f -> p c f", p=P),
                    )
                    v_piece.wv = wv_t
                wv_t = v_piece.wv
                jsl = slice(jc * P, (jc + 1) * P)
                vps = krp.tile([P, 512], FP32, tag="kr", name="vps")
                for kc in range(NCH):
                    nc.tensor.matmul(
                        vps, xT[:, kc, jsl], wv_t[:, kc, :],
                        start=(kc == 0), stop=(kc == NCH - 1),
                    )
                nc.scalar.copy(V[:, jc, sl], vps)

            for q in range(4):
                kr_quarter(0, q)
            for i in range(NHP + 2):
                if i < NHP:
                    scores_init(i)
                pieces = []
                if i < NHP:
                    pieces += [("S", i, 0)]
                if i + 1 < NHP:
                    pieces += [("K", i + 1, 0)]
                if i < NHP:
                    pieces += [("S", i, 1)]
                if i + 1 < NHP:
                    pieces += [("K", i + 1, 1)]
                if 1 <= i <= NHP:
                    pieces += [("T", i - 1, 0)]
                if i < NHP:
                    pieces += [("S", i, 2)]
                if i + 1 < NHP:
                    pieces += [("K", i + 1, 2)]
                if i >= 2:
                    pieces += [("A", i - 2, 0)]
                if i < NHP:
                    pieces += [("S", i, 3)]
                if i + 1 < NHP:
                    pieces += [("K", i + 1, 3)]
                if 1 <= i <= NHP:
                    pieces += [("T", i - 1, 1)]
                if i < 4:
                    pieces += [("V", i, jc) for jc in range(2)]
                for kind, a, b in pieces:
                    if kind == "S":
                        scores_ib(a, b)
                    elif kind == "K":
                        kr_quarter(a, b)
                    elif kind == "T":
                        transpose_hh(a, b)
                    elif kind == "A":
                        av_block(a)
                    elif kind == "V":
                        v_piece(a // 2, (a % 2) * 4 + b * 2)
                        v_piece(a // 2, (a % 2) * 4 + b * 2 + 1)

            # ---------------- Final projection ----------------
            ob_pool = ctx.enter_context(tc.tile_pool(name="ob", bufs=2))
            for eh in range(2):
                esl = slice(eh * 512, (eh + 1) * 512)
                wpj_t = wv_pool.tile([P, NCH, 512], BF16, tag="wv", name="wpj_t")
                nc.sync.dma_start(
                    wpj_t,
                    wproj[:, eh * 512:(eh + 1) * 512].rearrange(
                        "(c p) f -> p c f", p=P),
                )
                for ib in range(NIB):
                    isl = slice(ib * P, (ib + 1) * P)
                    fp = avp.tile([P, CUR], FP32, tag="av", name="fp")
                    for fc in range(NCH):
                        nc.tensor.matmul(
                            fp, AVT[:, fc, isl], wpj_t[:, fc, :],
                            start=(fc == 0), stop=(fc == NCH - 1),
                        )
                    o_ib = ob_pool.tile([P, 512], FP32, tag="ob", name="o_ib")
                    nc.vector.tensor_tensor(o_ib, fp, boutb[:, esl], ALU.add)
                    nc.sync.dma_start(
                        out[ib * P:(ib + 1) * P, eh * 512:(eh + 1) * 512],
                        o_ib,
                    )

    if split_waits:
        _split_multiwait(nc)
    return nc


def _get_nc():
    global _BUILT
    if _BUILT is None:
        _BUILT = _build()
    return _BUILT


def _prep_host(inputs, pos_embedding, full_input, u, v, mask,
               W_kv, b_kv, W_q, b_q, W_pos, b_pos, W_proj, b_proj):
    f32 = np.float32
    W_k = np.ascontiguousarray(W_kv[:, : H * D])
    W_v = np.ascontiguousarray(W_kv[:, H * D:])
    b_k = b_kv[: H * D].astype(f32)
    b_v = b_kv[H * D:].astype(f32)
    bias_qu = (b_q + u.ravel()).astype(f32)
    bias_dqv = (v.ravel() - u.ravel()).astype(f32)
    b_out = (b_v @ W_proj + b_proj).astype(f32)

    bias_all = np.stack(
        [bias_qu.reshape(NCH, P), bias_dqv.reshape(NCH, P),
         b_k.reshape(NCH, P), b_pos.astype(f32).reshape(NCH, P)], axis=0
    )  # [4, NCH, P]
    bias_all = np.ascontiguousarray(bias_all.transpose(2, 0, 1).reshape(P, 4 * NCH))
    wmats_np = np.concatenate([W_q, W_pos, W_k, W_v], axis=1).astype(nbf16)
    shared = {
        "wmats": wmats_np,
        "wproj": W_proj.astype(nbf16),
        "biases": bias_all.astype(f32),
        "bout": b_out.astype(f32),
    }
    pT_np = pos_embedding[:, 0].T
    in_maps = []
    for c in range(BS):
        m = dict(shared)
        m["acts"] = np.concatenate(
            [full_input[:, c].T, inputs[:, c].T, pT_np], axis=1
        ).astype(nbf16)
        in_maps.append(m)
    return in_maps


def kernel(**inputs):
    nc = _get_nc()
    in_maps = _prep_host(**{k: np.asarray(v) for k, v in inputs.items()})
    res = run_bass_kernel_spmd(nc, in_maps, list(range(BS)))
    out = np.stack([res.results[c]["out"] for c in range(BS)], axis=1)
    return np.ascontiguousarray(out.astype(np.float32))


if __name__ == "__main__":
    nc = _build()
    print("built ok")


# revision 25
# speedup vs baseline: 1.0265x; 1.0265x over previous
"""TransformerXL attention (AttentionXL) Bass kernel for Trainium2, 8 NeuronCores.

Sharding: pure data-parallel over batch (BS=8 -> 1 batch element per core).
All weights replicated per core; no collectives.

v2 design (multiplicative softmax split, unnormalized attention):
  exp(s*(C+S)) = exp(s*C) * exp(s*S):
    expC  = exp(s*C) on ScalarE straight from the C psum
    expS  = exp(s*P) on ScalarE (this replaces the fp32->bf16 cast of the
            position scores that the DMA rel-shift roundtrip needed anyway)
    A~    = expC (*) expS_sheared   fused with Z-rowsum on DVE
            (tensor_tensor_reduce, bf16 fast path)
  A~ stays unnormalized through transpose (PE) + AV; 1/Z is folded into the
  AVT psum->sbuf copy as a per-head broadcast multiply.
  Causal mask: fill expS with 0 on masked cols (gpsimd affine_select).

  C and P score matmuls are head-pair row-packed (tile_position (0,0)/(64,0),
  contraction 64 each -> concurrent).  P scores m-trimmed to m >= 384-128*ib.
  Projections K/R stream per head-pair chunk interleaved with attention so
  ScalarE/DVE softmax work overlaps PE projection work.
"""

import os
import sys

for _p in (
    "/root/.axon_site",
    "/root/.axon_site/_ro/trn_rl_repo",
    "/root/.axon_site/_ro/pypackages",
    "/opt/trn_rl_repo",
):
    if os.path.isdir(_p) and _p not in sys.path:
        sys.path.append(_p)

import numpy as np
import ml_dtypes

import concourse.bass as bass
import concourse.mybir as mybir
import concourse.tile as tile
from concourse.bass_utils import run_bass_kernel_spmd
from concourse.masks import make_identity

BF16 = mybir.dt.bfloat16
FP32 = mybir.dt.float32
AF = mybir.ActivationFunctionType
ALU = mybir.AluOpType
nbf16 = ml_dtypes.bfloat16

CUR, FULL, BS, DIM, H, D = 512, 1024, 8, 1024, 16, 64
PREV = FULL - CUR
SCALE = 1.0 / D**0.5
P = 128
NIB = CUR // P    # 4 query blocks
NJC = FULL // P   # 8 key chunks
NCH = DIM // P    # 8 dim chunks
NHP = H // 2      # 8 head pairs

_BUILT = None


def _split_multiwait(nc):
    """encode at most ONE sync wait per TPB instruction (single wait slot):
    prepend same-engine NoOps carrying extra waits."""
    n_split = 0
    for fn in nc.m.functions:
        for blk in fn.blocks:
            insts = list(blk.instructions)
            out = []
            for ins in insts:
                si = ins.sync_info
                if si is not None and si.on_wait and len(si.on_wait) > 1:
                    waits = list(si.on_wait)
                    for w in waits[:-1]:
                        nop = mybir.InstNoOp(
                            name=f"{ins.name}-ws{n_split}",
                            engine=ins.engine,
                            sync_info=mybir.SyncInfo(on_wait=[w], on_update=[]),
                            text_hint="waitsplit",
                        )
                        out.append(nop)
                        n_split += 1
                    ins.sync_info = mybir.SyncInfo(
                        on_wait=[waits[-1]],
                        on_update=list(si.on_update or []),
                    )
                out.append(ins)
            blk.instructions = out
    return n_split


def _build(split_waits=True):
    nc = bass.Bass()

    # activations transposed: [X^T | Xc^T | Pos^T] cols
    acts = nc.declare_dram_parameter("acts", [DIM, FULL + CUR + FULL], BF16, isOutput=False)
    # weights: [W_q | W_pos | W_k | W_v] cols
    wmats = nc.declare_dram_parameter("wmats", [DIM, 4 * DIM], BF16, isOutput=False)
    wproj = nc.declare_dram_parameter("wproj", [DIM, DIM], BF16, isOutput=False)
    # biases: [p, 4*NCH] = qu | dqv(=v-u) | k | pos chunks
    biases = nc.declare_dram_parameter("biases", [P, 4 * NCH], FP32, isOutput=False)
    bout = nc.declare_dram_parameter("bout", [DIM], FP32, isOutput=False)
    out = nc.declare_dram_parameter("out", [CUR, DIM], FP32, isOutput=True)

    with tile.TileContext(nc) as tc:
        from contextlib import ExitStack

        with ExitStack() as ctx:
            persist = ctx.enter_context(tc.tile_pool(name="persist", bufs=1))

            QuT = persist.tile([P, NCH, CUR], BF16, tag="QuT")
            QvT = persist.tile([P, NCH, CUR], BF16, tag="QvT")
            V = persist.tile([P, NJC, DIM], BF16, tag="V")
            AVT = persist.tile([P, NCH, CUR], BF16, tag="AVT")
            xcT = persist.tile([P, NCH, CUR], BF16, tag="xcT")
            xT = persist.tile([P, NCH, FULL], BF16, tag="xT")
            pT = persist.tile([P, NCH, FULL], BF16, tag="pT")
            bias_t = persist.tile([P, 4, NCH], FP32, tag="bias_t")
            boutb = persist.tile([P, DIM], FP32, tag="boutb")
            ident = persist.tile([P, P], BF16, tag="ident")

            make_identity(nc, ident)
            zero_reg = nc.gpsimd.to_reg(0.0)
            nc.sync.dma_start(bias_t, biases.rearrange("p (b c) -> p b c", b=4))
            # bout broadcast: replicate via DMA into all 128 partition rows
            nc.sync.dma_start(
                boutb,
                bout[None, :].broadcast_to((P, DIM)),
            )

            # ---- input DMAs, finest-grain-first so compute starts early ----
            nc.sync.dma_start(xcT, acts[:, FULL:FULL + CUR].rearrange("(c p) f -> p c f", p=P))

            # weight streaming pools
            wq_pool = ctx.enter_context(tc.tile_pool(name="wq", bufs=2))
            wkr_pool = ctx.enter_context(tc.tile_pool(name="wkr", bufs=2))
            wv_pool = ctx.enter_context(tc.tile_pool(name="wv", bufs=2))
            ktr_pool = ctx.enter_context(tc.tile_pool(name="ktr", bufs=2))
            qv_pool = None  # QvT persistent

            # psum pools
            scores = ctx.enter_context(tc.tile_pool(name="scores", bufs=2, space="PSUM"))
            krp = ctx.enter_context(tc.tile_pool(name="krp", bufs=2, space="PSUM"))
            tps = ctx.enter_context(tc.tile_pool(name="tps", bufs=1, space="PSUM"))
            avp = ctx.enter_context(tc.tile_pool(name="avp", bufs=1, space="PSUM"))

            # attention work pools
            work = ctx.enter_context(tc.tile_pool(name="work", bufs=1))
            dram = ctx.enter_context(tc.tile_pool(name="dram", bufs=4, space="DRAM"))

            # ---------------- Q projection ----------------
            for oc in range(NCH):
                wq_t = wq_pool.tile([P, NCH, P], BF16, tag="wq", name="wq_t")
                nc.sync.dma_start(
                    wq_t,
                    wmats[:, oc * P:(oc + 1) * P].rearrange("(c p) f -> p c f", p=P),
                )
                qps = krp.tile([P, 512], FP32, tag="kr", name="qps")
                for kc in range(NCH):
                    nc.tensor.matmul(
                        qps, wq_t[:, kc, :], xcT[:, kc, :],
                        start=(kc == 0), stop=(kc == NCH - 1),
                    )
                nc.scalar.activation(
                    QuT[:, oc, :], qps, AF.Identity,
                    bias=bias_t[:, 0, oc:oc + 1],
                )
                nc.gpsimd.tensor_scalar_add(
                    QvT[:, oc, :], QuT[:, oc, :], bias_t[:, 1, oc:oc + 1]
                )

            nc.sync.dma_start(xT, acts[:, 0:FULL].rearrange("(c p) f -> p c f", p=P))
            nc.sync.dma_start(
                pT, acts[:, FULL + CUR:].rearrange("(c p) f -> p c f", p=P))

            # ---- software-pipelined head-pair loop ----
            # iteration i: scores(i) [C/P mm + exp + shear-dma + mask + STT],
            #              K/R proj(i+1), V chunk (i<2), transposes(i-1), AV(i-2)
            KR = {}
            ST = {}
            AT = {}

            def proj_kr(hp):
                KT_t = ktr_pool.tile([P, FULL], BF16, tag="kt", name="KT_t")
                RT_t = ktr_pool.tile([P, FULL], BF16, tag="rt", name="RT_t")
                wk_t = wkr_pool.tile([P, NCH, P], BF16, tag="wk", name="wk_t")
                wp_t = wkr_pool.tile([P, NCH, P], BF16, tag="wp", name="wp_t")
                nc.sync.dma_start(
                    wk_t,
                    wmats[:, 2 * DIM + hp * P:2 * DIM + (hp + 1) * P].rearrange(
                        "(c p) f -> p c f", p=P),
                )
                nc.sync.dma_start(
                    wp_t,
                    wmats[:, DIM + hp * P:DIM + (hp + 1) * P].rearrange(
                        "(c p) f -> p c f", p=P),
                )
                for jh in range(2):
                    sl = slice(jh * 512, (jh + 1) * 512)
                    kps = krp.tile([P, 512], FP32, tag="kr", name="kps")
                    for kc in range(NCH):
                        nc.tensor.matmul(
                            kps, wk_t[:, kc, :], xT[:, kc, sl],
                            start=(kc == 0), stop=(kc == NCH - 1),
                        )
                    nc.scalar.activation(KT_t[:, sl], kps, AF.Identity,
                                         bias=bias_t[:, 2, hp:hp + 1])
                for jh in range(2):
                    sl = slice(jh * 512, (jh + 1) * 512)
                    rps = krp.tile([P, 512], FP32, tag="kr", name="rps")
                    for kc in range(NCH):
                        nc.tensor.matmul(
                            rps, wp_t[:, kc, :], pT[:, kc, sl],
                            start=(kc == 0), stop=(kc == NCH - 1),
                        )
                    nc.vector.tensor_scalar_add(RT_t[:, sl], rps,
                                                bias_t[:, 3, hp:hp + 1])
                KR[hp] = (KT_t, RT_t)

            def scores_block(hp):
                KT_t, RT_t = KR.pop(hp)
                pd = [dram.tile([CUR, FULL], BF16, tag=f"pd{hh}", name=f"pd{hh}")
                      for hh in range(2)]
                zc = work.tile([P, 8], FP32, tag="zc", bufs=2, name="zc")
                a_sbs = {}
                for ib in range(NIB):
                    isl = slice(ib * P, (ib + 1) * P)
                    jmax = 640 + ib * P
                    mlo = PREV - P - ib * P
                    cps_, pps_, acs, ss = [], [], [], []
                    for hh in range(2):
                        cps_.append(scores.tile([P, FULL], FP32, tag="sc", name="cp"))
                    for jh in range(2):
                        j0, j1 = jh * 512, min((jh + 1) * 512, jmax)
                        for hh in range(2):
                            rs = slice(hh * D, (hh + 1) * D)
                            nc.tensor.matmul(
                                cps_[hh][:, j0:j1],
                                QuT[rs, hp, isl], KT_t[rs, j0:j1],
                                start=True, stop=True,
                                tile_position=(hh * D, 0),
                            )
                    for hh in range(2):
                        a_c = work.tile([P, FULL], BF16, tag="ac", bufs=4, name="a_c")
                        nc.scalar.activation(
                            a_c[:, 0:jmax], cps_[hh][:, 0:jmax], AF.Exp, scale=SCALE,
                        )
                        acs.append(a_c)
                    for hh in range(2):
                        pps_.append(scores.tile([P, FULL], FP32, tag="sc", name="pp"))
                    for mh in range(2):
                        m0, m1 = max(mh * 512, mlo), (mh + 1) * 512
                        if m1 <= m0:
                            continue
                        for hh in range(2):
                            rs = slice(hh * D, (hh + 1) * D)
                            nc.tensor.matmul(
                                pps_[hh][:, m0:m1],
                                QvT[rs, hp, isl], RT_t[rs, m0:m1],
                                start=True, stop=True,
                                tile_position=(hh * D, 0),
                            )
                    for hh in range(2):
                        p_sb = work.tile([P, FULL], BF16, tag="psb", bufs=3, name="p_sb")
                        nc.scalar.activation(
                            p_sb[:, mlo:], pps_[hh][:, mlo:], AF.Exp, scale=SCALE,
                        )
                        nc.sync.dma_start(
                            bass.AP(tensor=pd[hh].tensor,
                                    offset=pd[hh].offset + ib * P * FULL + mlo,
                                    ap=[[FULL, P], [1, FULL - mlo]]),
                            p_sb[:, mlo:],
                        )
                        s_sb = work.tile([P, FULL], BF16, tag="ssb", bufs=4, name="s_sb")
                        nc.sync.dma_start(
                            s_sb[:, 0:jmax],
                            bass.AP(tensor=pd[hh].tensor,
                                    offset=pd[hh].offset + ib * P * (FULL - 1) + (PREV - 1),
                                    ap=[[FULL - 1, P], [1, jmax]]),
                        )
                        nc.gpsimd.affine_select(
                            out=s_sb[:, 512:jmax], in_=s_sb[:, 512:jmax],
                            compare_op=ALU.is_ge, fill=zero_reg,
                            base=ib * P, channel_multiplier=1,
                            pattern=[[-1, jmax - 512]],
                        )
                        ss.append(s_sb)
                    for hh in range(2):
                        a_sb = work.tile([P, 640 + 128 * ib], BF16,
                                         tag=f"asb{ib}", bufs=4, name="a_sb")
                        nc.vector.scalar_tensor_tensor(
                            out=a_sb, in0=acs[hh][:, 0:jmax],
                            scalar=1.0, in1=ss[hh][:, 0:jmax],
                            op0=ALU.mult, op1=ALU.mult,
                            accum_out=zc[:, hh * 4 + ib:hh * 4 + ib + 1],
                        )
                        a_sbs[(hh, ib)] = a_sb
                ST[hp] = (a_sbs, zc)

            def transpose_block(hp):
                a_sbs, zc = ST.pop(hp)
                a_t = [work.tile([P, NJC, CUR], BF16, tag=f"at{hh}", bufs=2,
                                 name=f"at{hh}") for hh in range(2)]
                rz8 = work.tile([P, 8], FP32, tag="rz8", bufs=2, name="rz8")
                nc.vector.reciprocal(rz8, zc)
                for ib in range(NIB):
                    isl = slice(ib * P, (ib + 1) * P)
                    njc_v = min(ib + 5, NJC)
                    for hh in range(2):
                        a_sb = a_sbs[(hh, ib)]
                        nc.vector.tensor_scalar_mul(
                            a_sb, a_sb, rz8[:, hh * 4 + ib:hh * 4 + ib + 1]
                        )
                        for tg in range(2):
                            jcs = [j for j in range(tg * 4, min((tg + 1) * 4, njc_v))]
                            if not jcs:
                                continue
                            tp = tps.tile([P, 4, P], BF16, tag="tp", name="tp")
                            for k, jc in enumerate(jcs):
                                nc.tensor.transpose(
                                    tp[:, k], a_sb[:, jc * P:(jc + 1) * P], ident
                                )
                            nc.vector.tensor_copy(
                                a_t[hh][:, jcs[0]:jcs[0] + len(jcs), isl],
                                tp[:, :len(jcs)],
                            )
                AT[hp] = a_t

            def av_block(hp):
                at_pair = AT.pop(hp)
                av = avp.tile([P, CUR], FP32, tag="av", name="av")
                for jc in range(NJC):
                    ilo = max(0, (jc - 4)) * P
                    for hh in range(2):
                        h = 2 * hp + hh
                        nc.tensor.matmul(
                            av[hh * D:(hh + 1) * D, ilo:],
                            V[:, jc, h * D:(h + 1) * D],
                            at_pair[hh][:, jc, ilo:],
                            start=(jc == 0), stop=(jc == NJC - 1),
                            tile_position=(0, hh * D),
                        )
                nc.vector.tensor_copy(AVT[:, hp, :], av)

            def v_chunk(quarter):
                mh, jhalf = quarter // 2, quarter % 2
                sl = slice(mh * 512, (mh + 1) * 512)
                if jhalf == 0:
                    wv_t = wv_pool.tile([P, NCH, 512], BF16, tag="wv", name="wv_t")
                    nc.sync.dma_start(
                        wv_t,
                        wmats[:, 3 * DIM + mh * 512:3 * DIM + (mh + 1) * 512].rearrange(
                            "(c p) f -> p c f", p=P),
                    )
                    v_chunk.wv = wv_t
                wv_t = v_chunk.wv
                for jp in range(jhalf * 2, jhalf * 2 + 2):
                    for half in range(2):
                        jc = jp * 2 + half
                        jsl = slice(jc * P, (jc + 1) * P)
                        vps = krp.tile([P, 512], FP32, tag="kr", name="vps")
                        for kc in range(NCH):
                            nc.tensor.matmul(
                                vps, xT[:, kc, jsl], wv_t[:, kc, :],
                                start=(kc == 0), stop=(kc == NCH - 1),
                            )
                        nc.scalar.copy(V[:, jc, sl], vps)

            proj_kr(0)
            for i in range(NHP + 2):
                if i < NHP:
                    scores_block(i)
                if i + 1 < NHP:
                    proj_kr(i + 1)
                if i < 4:
                    v_chunk(i)
                if 1 <= i <= NHP:
                    transpose_block(i - 1)
                if i >= 2:
                    av_block(i - 2)

            # ---------------- Final projection ----------------
            ob_pool = ctx.enter_context(tc.tile_pool(name="ob", bufs=2))
            for eh in range(2):
                esl = slice(eh * 512, (eh + 1) * 512)
                wpj_t = wv_pool.tile([P, NCH, 512], BF16, tag="wv", name="wpj_t")
                nc.sync.dma_start(
                    wpj_t,
                    wproj[:, eh * 512:(eh + 1) * 512].rearrange(
                        "(c p) f -> p c f", p=P),
                )
                for ib in range(NIB):
                    isl = slice(ib * P, (ib + 1) * P)
                    fp = avp.tile([P, CUR], FP32, tag="av", name="fp")
                    for fc in range(NCH):
                        nc.tensor.matmul(
                            fp, AVT[:, fc, isl], wpj_t[:, fc, :],
                            start=(fc == 0), stop=(fc == NCH - 1),
                        )
                    o_ib = ob_pool.tile([P, 512], FP32, tag="ob", name="o_ib")
                    nc.vector.tensor_tensor(o_ib, fp, boutb[:, esl], ALU.add)
                    nc.sync.dma_start(
                        out[ib * P:(ib + 1) * P, eh * 512:(eh + 1) * 512],
                        o_ib,
                    )

    if split_waits:
        _split_multiwait(nc)
    return nc


def _get_nc():
    global _BUILT
    if _BUILT is None:
        _BUILT = _build()
    return _BUILT


def _prep_host(inputs, pos_embedding, full_input, u, v, mask,
               W_kv, b_kv, W_q, b_q, W_pos, b_pos, W_proj, b_proj):
    f32 = np.float32
    W_k = np.ascontiguousarray(W_kv[:, : H * D])
    W_v = np.ascontiguousarray(W_kv[:, H * D:])
    b_k = b_kv[: H * D].astype(f32)
    b_v = b_kv[H * D:].astype(f32)
    bias_qu = (b_q + u.ravel()).astype(f32)
    bias_dqv = (v.ravel() - u.ravel()).astype(f32)
    b_out = (b_v @ W_proj + b_proj).astype(f32)

    bias_all = np.stack(
        [bias_qu.reshape(NCH, P), bias_dqv.reshape(NCH, P),
         b_k.reshape(NCH, P), b_pos.astype(f32).reshape(NCH, P)], axis=0
    )  # [4, NCH, P]
    bias_all = np.ascontiguousarray(bias_all.transpose(2, 0, 1).reshape(P, 4 * NCH))
    wmats_np = np.concatenate([W_q, W_pos, W_k, W_v], axis=1).astype(nbf16)
    shared = {
        "wmats": wmats_np,
        "wproj": W_proj.astype(nbf16),
        "biases": bias_all.astype(f32),
        "bout": b_out.astype(f32),
    }
    pT_np = pos_embedding[:, 0].T
    in_maps = []
    for c in range(BS):
        m = dict(shared)
        m["acts"] = np.concatenate(
            [full_input[:, c].T, inputs[:, c].T, pT_np], axis=1
        ).astype(nbf16)
        in_maps.append(m)
    return in_maps


def kernel(**inputs):
    nc = _get_nc()
    in_maps = _prep_host(**{k: np.asarray(v) for k, v in inputs.items()})
    res = run_bass_kernel_spmd(nc, in_maps, list(range(BS)))
    out = np.stack([res.results[c]["out"] for c in range(BS)], axis=1)
    return np.ascontiguousarray(out.astype(np.float32))


if __name__ == "__main__":
    nc = _build()
    print("built ok")
